# revision 71
# baseline (speedup 1.0000x reference)
"""Linear-chain CRF forward pass on 8 Trainium2 NeuronCores.

Reference recurrence (per batch element b):
    alpha_t[j] = x_t[j] + logsumexp_k(alpha_{t-1}[k] + trans[j,k])
    out[b] = sum_j alpha_{L_b - 1}[j]

Exp-space device formulation with a constant per-step log shift c folded
into the transition matrix:
    E_t = (Mc @ E_{t-1}) * X_t,  Mc[j,k] = exp(trans[j,k] - c),  X_t = exp(x_t)

The T=2048-step serial chain is cut per batch element into a chain of
segments with boundaries on multiples of 8; each segment evolves
independently from a raw X init (warmup W=2 inside the previous segment's
coverage; Birkhoff contraction converges the direction) and the per-segment
log offsets are recovered on the host by telescoping class-mean log-ratios
at the boundaries (each segment's end-state snapshot vs the host-known raw
init of the next segment).

Step 1 is computed on the host in fp32 (rawa ships E_1 = (Mc@X_0)*X_1),
so device round r advances step r+1 and only 16 rounds (0..15) run on
device.  Two segment populations share the schedule:
  - d-segments (16 steps, 1 step/round) live in the death-sorted front
    region: PE matmul -> PSUM fp32, DVE multiplies by X straight out of
    PSUM (2 chains).  Extraction segments and seg0 are always d-type.
  - hop-segments (8 steps, 1 step per TWO rounds) live in two YP-col
    suffix regions: PE matmul -> PSUM, ACT copies PSUM -> SBUF, GPSIMD
    (Pool) tensor_mul by X (neuronxcc rejects TensorScalarPtr on Pool).
    Chain A steps on even rounds, chain B on odd rounds; the 2-round
    cadence gives the PE->ACT->Pool serial path two full rounds, so it
    never stalls the d-chains.
  - Only live columns are shipped / computed: columns are death-sorted so
    the alive set each round is a suffix; extraction columns die at their
    extraction round (max 9), seg0 at its boundary snapshot (6 or 14).
  - Batch elements are grouped by extraction round and dealt round-robin
    to cores, so all cores share one extraction schedule and workload.
  - X = exp(x) is precomputed on the host, shipped bf16 in per-round
    variable-width slabs chunked through a 4-deep SBUF ring.
"""

from contextlib import ExitStack

import numpy as np

B, T, C = 256, 2048, 64
NCORES = 8
BPC = B // NCORES          # 32
SL = 16                    # d-segment steps
HS = 8                     # hop-segment steps (one step per 2 rounds)
W = 2                      # warmup rounds
L = SL                     # rounds 0..15; device round r computes step r+1
LB = SL                    # (step 1 = E1 is computed on the host)
RB = HS - W - 1            # seg0 bridge snapshot ROUND (step 6 at round 5)
RA = SL - W - 1            # seg0 snapshot ROUND (step 14 at round 13)
SNAPB = HS - W             # seg0 snapshot when first gap is 8
SNAPA = SL - W             # seg0 snapshot when first gap is 16
ED = 7                     # et ring depth
DCH = 2                    # rounds per X DMA chunk
RING = 4
NSX = 3
YP = 370                   # cols per hop chain (2 chains)
PRE_FILL = 5
FILLERS = 2

_CACHE = {}


def _c_step(transitions, pad_x):
    """Mean per-step growth of max_j alpha, from a short host simulation."""
    x = np.asarray(pad_x[:4], np.float64)
    tr = np.asarray(transitions, np.float64)
    a = x[:, 0, :]
    tot, n = 0.0, 0
    for t in range(1, 257):
        s = a[:, None, :] + tr[None, :, :]
        m = s.max(axis=2, keepdims=True)
        a_new = x[:, t, :] + np.log(np.exp(s - m).sum(axis=2)) + m[:, :, 0]
        tot += float((a_new.max(axis=1) - a.max(axis=1)).mean())
        n += 1
        a = a_new
    return tot / n


class _Plan:
    pass


def _plan(batch_sizes):
    bs = np.asarray(batch_sizes).astype(np.int64)
    p = _Plan()

    # --- assignment: group by r_e, round-robin to cores -------------------
    info = []
    for b in range(B):
        ts = int(bs[b]) - 1
        if ts < HS:
            r_e, bm = ts, 0
        else:
            bm = (ts // HS) * HS
            r_e = ts - bm + W
        info.append((r_e, ts, b, bm))
    info.sort()
    p.gidx = np.zeros((NCORES, BPC), np.int64)
    p.re = np.zeros((NCORES, BPC), np.int64)
    p.tstar = np.zeros((NCORES, BPC), np.int64)
    p.bm = np.zeros((NCORES, BPC), np.int64)
    for rank, (r_e, ts, b, bm) in enumerate(info):
        k, e = rank % NCORES, rank // NCORES
        p.gidx[k, e] = b
        p.re[k, e] = r_e
        p.tstar[k, e] = ts
        p.bm[k, e] = bm
    p.cre = np.maximum(p.re - 1, 0)      # device copy round per event
    ext_death = p.cre.max(axis=0)
    assert int(ext_death.max()) <= HS + W - 1

    # --- per-element segment gap lists (8s and 16s), hop quota ------------
    QHOP = 4 * YP              # hop half-slots per core (2 chains x 2 halves)
    p.gaps = [[None] * BPC for _ in range(NCORES)]
    counts = {"B0": np.zeros(NCORES, np.int64),
              "A0": np.zeros(NCORES, np.int64),
              "D": np.zeros(NCORES, np.int64),
              "H": np.zeros(NCORES, np.int64)}
    for k in range(NCORES):
        q = QHOP
        # big elements first so quota parity always resolves
        order = sorted(range(BPC), key=lambda e: -int(p.bm[k, e]))
        for e in order:
            G = int(p.bm[k, e]) // HS
            # seg0's gap is free (B0/A0 column); only mid 8-gaps use hop
            # slots, so up to q+1 eights fit.  Parity: n8 must match G.
            n8 = min(G, q + 1)
            if (n8 - G) % 2:
                n8 -= 1
            if n8 < G % 2:
                n8 = G % 2
            q -= max(n8 - 1, 0)
            n16 = (G - n8) // 2
            # gap list in chain order: one 8 first if any (seg0 -> B0)
            if n8 >= 1:
                gaps = [8] * n8 + [16] * n16
            else:
                gaps = [16] * n16
            assert sum(gaps) == int(p.bm[k, e])
            p.gaps[k][e] = gaps
            if gaps:
                counts["B0" if gaps[0] == 8 else "A0"][k] += 1
                counts["H"][k] += (n8 - 1) if n8 >= 1 else 0
                counts["D"][k] += n16 if n8 >= 1 else n16 - 1
        assert q >= -1
    NB0 = int(max((int(n) + 1) // 2 for n in counts["B0"]))
    NA0 = int(max((int(n) + 1) // 2 for n in counts["A0"]))
    NDM = int(max((int(n) + 1) // 2 for n in counts["D"])) + 1

    # --- global death-sorted d-region columns -----------------------------
    cols = [(int(ext_death[e]), 0, e) for e in range(BPC)]
    cols += [(RB, 1, i) for i in range(NB0)]
    cols.sort()
    cols += [(RA, 2, i) for i in range(NA0)]
    cols += [(SL - 1, 3, i) for i in range(NDM)]
    p.zs = len(cols)
    p.ncol = p.zs + 2 * YP
    p.col_death = np.array([cc[0] for cc in cols], np.int64)
    assert np.all(np.diff(p.col_death) >= 0)
    p.ext_col = np.zeros(BPC, np.int64)
    b0_cols, a0_cols = [], []
    dm0 = None
    for ci, (_, cls, ident) in enumerate(cols):
        if cls == 0:
            p.ext_col[ident] = ci
        elif cls == 1:
            b0_cols.append(ci)
        elif cls == 2:
            a0_cols.append(ci)
        elif dm0 is None:
            dm0 = ci
    p.b0_rng = (b0_cols[0], b0_cols[-1] + 1) if b0_cols else (0, 0)
    p.a0_rng = (a0_cols[0], a0_cols[-1] + 1) if a0_cols else (0, 0)
    p.dm0 = dm0 if dm0 is not None else p.zs

    p.d = np.array([int(np.searchsorted(p.col_death, r))
                    for r in range(L + 1)], np.int64)

    # --- per-core slot assignment ----------------------------------------
    # segs[k][e] = list of (t0, steps, kind, col, half); ext is separate
    p.segs = [[None] * BPC for _ in range(NCORES)]
    for k in range(NCORES):
        it_b0 = iter([(c, h) for c in b0_cols for h in (0, 1)])
        it_a0 = iter([(c, h) for c in a0_cols for h in (0, 1)])
        it_d = iter([(c, h) for c in range(p.dm0, p.zs) for h in (0, 1)])
        it_h = iter([(c, h) for c in range(p.zs, p.ncol) for h in (0, 1)])
        for e in range(BPC):
            gaps = p.gaps[k][e]
            segs = []
            b_cum = 0
            for j, g in enumerate(gaps):
                t0 = 0 if j == 0 else b_cum - W
                if j == 0:
                    kind = "B0" if g == 8 else "A0"
                    col, half = next(it_b0 if g == 8 else it_a0)
                    steps = SNAPB if g == 8 else SNAPA
                elif g == 16:
                    kind, (col, half), steps = "D", next(it_d), SL
                else:
                    kind, (col, half), steps = "H", next(it_h), HS
                segs.append((t0, steps, kind, col, half))
                b_cum += g
            p.segs[k][e] = segs

    # --- extraction copy ops ---------------------------------------------
    re_min = p.cre.min(axis=0)
    re_max = p.cre.max(axis=0)
    p.copies = []
    fin_off = 0
    for r in range(L):
        es = [e for e in range(BPC) if re_min[e] <= r <= re_max[e]]
        if not es:
            continue
        runs = []
        for e in es:
            cc = int(p.ext_col[e])
            if runs and e == runs[-1][1] + 1 and cc == runs[-1][3] + 1:
                runs[-1][1] = e
                runs[-1][3] = cc
            else:
                runs.append([e, e, cc, cc])
        for (e0, e1, c0, _c1) in runs:
            n = e1 - e0 + 1
            p.copies.append((r, e0, n, c0, fin_off))
            fin_off += n
    p.nfin = fin_off
    p.fincol = np.zeros((NCORES, BPC), np.int64)
    for k in range(NCORES):
        for e in range(BPC):
            r = int(p.cre[k, e])
            for (rr, e0, n, c0, fa) in p.copies:
                if rr == r and e0 <= e < e0 + n:
                    p.fincol[k, e] = fa + (e - e0)
                    break
            else:
                raise AssertionError("no copy op for event")
    p.cum_copies = np.zeros(L + 1, np.int64)
    for r in range(L):
        p.cum_copies[r + 1] = p.cum_copies[r] + sum(
            1 for (rr, *_x) in p.copies if rr == r)

    # --- chain geometry ---------------------------------------------------
    # fixed split point: chain 1 = [MID, zs) constant; chain 0 absorbs the
    # shrinking alive front [d(r), MID) (ranges must nest across rounds)
    MID = (int(p.d[L - 1]) + p.zs) // 2
    p.mids = np.zeros(L, np.int64)
    for r in range(1, L):
        lo = int(p.d[r])
        assert MID - lo <= 512 and p.zs - MID <= 512
        p.mids[r] = MID
    for (r, e0, n, c0, fa) in p.copies:
        if r >= 1:
            assert c0 + n <= int(p.mids[r])
    assert p.b0_rng[1] <= int(p.mids[RB]) or p.b0_rng[1] == 0

    # --- X slab layout ----------------------------------------------------
    # round r slab: d-part [d(r), zs) + hop part (512 cols) for the active
    # chain: A steps on even rounds (2..16, step r/2), B on odd (1..15,
    # step (r+1)/2).  B's step 8 lands at round 15 so its end snapshot
    # ships during round 16.
    def hop_chain(r):
        if 2 <= r <= 2 * (HS - 1) and r % 2 == 0:
            return 0
        if 1 <= r <= 2 * (HS - 1) - 1 and r % 2 == 1:
            return 1
        return None
    p.hop_chain = hop_chain
    p.O = np.zeros(LB + 1, np.int64)
    p.O[1] = p.ncol                      # rawa
    for r in range(1, LB):
        wdt = p.zs - int(p.d[r])
        if hop_chain(r) is not None:
            wdt += YP
        p.O[r + 1] = p.O[r] + wdt
    p.ntot = int(p.O[LB])
    CB = [1, 2, 3]
    while CB[-1] < LB:
        CB.append(min(CB[-1] + DCH, LB))
    p.CB = CB
    p.nchunk = len(CB) - 1
    p.chunk_of = [0] * LB
    for kk in range(p.nchunk):
        for r in range(CB[kk], CB[kk + 1]):
            p.chunk_of[r] = kk
    p.maxchunkw = max(int(p.O[CB[kk + 1]] - p.O[CB[kk]])
                      for kk in range(p.nchunk))

    # early/late fin split
    p.fin_split = 0
    p.fin_ops_early = 0
    for (r, e0, n, c0, fa) in p.copies:
        if r <= HS - 2:
            p.fin_ops_early += 1
            p.fin_split = max(p.fin_split, fa + n)
    if p.fin_split > p.nfin - 4:
        p.fin_split = 0
    return p


def _build_host_inputs(p, pad_x, transitions, origination, c):
    import ml_dtypes
    mc = np.exp(np.asarray(transitions, np.float64) - c).astype(np.float32)
    wmat = np.zeros((128, 128), ml_dtypes.bfloat16)
    wmat[:64, :64] = mc.T.astype(ml_dtypes.bfloat16)
    wmat[64:, 64:] = mc.T.astype(ml_dtypes.bfloat16)

    x0 = np.asarray(pad_x, np.float32).copy()
    x0[:, 0, :] += np.asarray(origination, np.float32)[None, :]

    ncol, zs = p.ncol, p.zs
    xraw = np.empty((NCORES, 128, p.ntot), ml_dtypes.bfloat16)
    x0s = np.empty((NCORES, 128, ncol), np.float32)
    for k in range(NCORES):
        t0s = np.full((2, ncol), -10 ** 9, np.int64)
        bofs = np.zeros((2, ncol), np.int64)
        for e in range(BPC):
            gb = int(p.gidx[k, e])
            ts = int(p.tstar[k, e])
            t0s[0, int(p.ext_col[e])] = 0 if ts < HS else int(p.bm[k, e]) - W
            bofs[0, int(p.ext_col[e])] = gb
            for (t0, steps, kind, col, half) in p.segs[k][e]:
                t0s[half, col] = t0
                bofs[half, col] = gb
        # device steps per column: d-region cols see step r at round r;
        # hop cols see step s at round 2s (A) / 2s+1 (B).
        xfull = np.empty((128, SL + 1, ncol), np.float32)
        for hh in range(2):
            t_idx = t0s[hh][:, None] + np.arange(SL + 1)[None, :]
            valid = (t_idx >= 0) & (t_idx < T) & (t0s[hh][:, None] > -10**8)
            t_clip = np.clip(t_idx, 0, T - 1)
            blk = x0[bofs[hh][:, None], t_clip, :]
            blk = np.where(valid[:, :, None], blk, 0.0)
            xfull[64 * hh:64 * hh + 64] = np.exp(blk).transpose(2, 1, 0)
        # host-computed step 1: E1 = (Mc @ X0) * X1 per 64-class half
        e1 = np.empty((128, ncol), np.float32)
        for hh in range(2):
            sl_ = slice(64 * hh, 64 * hh + 64)
            e1[sl_] = (mc @ xfull[sl_, 0, :]) * xfull[sl_, 1, :]
        out = xraw[k]
        out[:, 0:ncol] = e1.astype(ml_dtypes.bfloat16)
        x0s[k] = xfull[:, 0, :]
        for r in range(1, LB):
            o = int(p.O[r])
            d = int(p.d[r])
            out[:, o:o + zs - d] = \
                xfull[:, r + 1, d:zs].astype(ml_dtypes.bfloat16)
            o += zs - d
            hc = p.hop_chain(r)
            if hc is not None:
                s = (r // 2 if hc == 0 else (r + 1) // 2) + 1
                cb = zs + hc * YP
                out[:, o:o + YP] = \
                    xfull[:, s, cb:cb + YP].astype(ml_dtypes.bfloat16)
    return xraw, wmat, x0s


def _build_program(p):
    import concourse.bass as bass
    from concourse import mybir

    dt = mybir.dt
    ncol, zs = p.ncol, p.zs
    CB = p.CB

    nc = bass.Bass()
    xp = nc.declare_dram_parameter("xp", [128, p.ntot], dt.bfloat16, False)
    wm = nc.declare_dram_parameter("wm", [128, 128], dt.bfloat16, False)
    snapb = nc.declare_dram_parameter(
        "snapb", [128, max(p.b0_rng[1] - p.b0_rng[0], 1)], dt.bfloat16, True)
    snapa = nc.declare_dram_parameter(
        "snapa", [128, max(p.a0_rng[1] - p.a0_rng[0], 1)], dt.bfloat16, True)
    snapf = nc.declare_dram_parameter(
        "snapf", [128, zs + YP - p.dm0], dt.bfloat16, True)
    snapfb = nc.declare_dram_parameter(
        "snapfb", [128, YP], dt.bfloat16, True)
    fin = nc.declare_dram_parameter("fin", [64, p.nfin], dt.bfloat16, True)

    with ExitStack() as ctx:
        def sb(name, shape, d):
            return ctx.enter_context(nc.sbuf_tensor(name, shape, d))
        w = sb("w", [128, 128], dt.bfloat16)
        rawa = sb("rawa", [128, ncol], dt.bfloat16)
        raw = [sb(f"raw{i}", [128, p.maxchunkw], dt.bfloat16)
               for i in range(RING)]
        et = [sb(f"et{i}", [128, ncol], dt.bfloat16) for i in range(ED)]
        hbp = [sb(f"hbp{i}", [128, YP], dt.bfloat16) for i in range(2)]
        fin_t = sb("fin_t", [64, p.nfin], dt.bfloat16)
        psd = [ctx.enter_context(
            nc.psum_tensor(f"psd{cidx}", [128, 512], dt.float32))
            for cidx in range(2)]
        psp = [ctx.enter_context(
            nc.psum_tensor(f"psp{i}", [128, 512], dt.float32))
            for i in range(2)]
        psf = ctx.enter_context(nc.psum_tensor("psf", [128, 128], dt.float32))
        s_w = ctx.enter_context(nc.semaphore("s_w"))
        s_xa = ctx.enter_context(nc.semaphore("s_xa"))
        s_xh = ctx.enter_context(nc.semaphore("s_xh"))
        s_x = tuple(ctx.enter_context(nc.semaphore(f"s_x{i}"))
                    for i in range(NSX))
        s_pd = ctx.enter_context(nc.semaphore("s_pd"))
        s_pp = ctx.enter_context(nc.semaphore("s_pp"))
        s_hp = ctx.enter_context(nc.semaphore("s_hp"))
        s_vd = ctx.enter_context(nc.semaphore("s_vd"))
        s_vp = ctx.enter_context(nc.semaphore("s_vp"))
        s_f = ctx.enter_context(nc.semaphore("s_f"))
        s_o = ctx.enter_context(nc.semaphore("s_o"))
        block = ctx.enter_context(nc.Block())

        def drng(r):
            lo, m = int(p.d[r]), int(p.mids[r])
            return ((lo, m - lo), (m, zs - m))

        def xsl_d(r, c0, n):
            kk = p.chunk_of[r]
            off = int(p.O[r] - p.O[CB[kk]]) + (c0 - int(p.d[r]))
            return raw[kk % RING][:, off:off + n]

        def xsl_h(r, hc):
            kk = p.chunk_of[r]
            off = int(p.O[r] - p.O[CB[kk]]) + \
                ((zs - int(p.d[r])) if r <= SL else 0)
            return raw[kk % RING][:, off:off + YP]

        def chunk_arrived(eng, r):
            kk = p.chunk_of[r]
            eng.wait_ge(s_x[kk % NSX], 16 * (kk // NSX + 1))

        @block.sync
        def _(sync):
            sync.dma_start(w[:], wm[:, :]).then_inc(s_w, 16)
            sync.dma_start(rawa[:], xp[:, 0:ncol]).then_inc(s_xa, 16)
            # chunk 0 d-part first; its hop part is deferred past chunk 1
            # (round-1 pool work has 2 rounds of slack, round-2 d does not)
            c0a, c0b = int(p.O[1]), int(p.O[2])
            wd1 = zs - int(p.d[1])
            sync.dma_start(raw[0][:, :wd1],
                           xp[:, c0a:c0a + wd1]).then_inc(s_x[0], 16)
            o12, o13 = int(p.O[2]), int(p.O[3])
            sync.dma_start(raw[1][:, :o13 - o12],
                           xp[:, o12:o13]).then_inc(s_x[1], 16)
            sync.dma_start(raw[0][:, wd1:c0b - c0a],
                           xp[:, c0a + wd1:c0b]).then_inc(s_xh, 16)
            for kk in range(2, p.nchunk):
                if kk >= RING:
                    r_last = CB[kk - RING + 1] - 1
                    sync.wait_ge(s_vd, 2 * min(r_last, SL - 1))
                    sync.wait_ge(s_vp, min(r_last, 2 * (HS - 1)))
                if kk >= NSX:
                    sync.wait_ge(s_x[kk % NSX], 16 * (kk // NSX))
                o0, o1 = int(p.O[CB[kk]]), int(p.O[CB[kk + 1]])
                sync.dma_start(
                    raw[kk % RING][:, :o1 - o0], xp[:, o0:o1],
                ).then_inc(s_x[kk % NSX], 16)
                if CB[kk] <= RB + 6 < CB[kk + 1] and \
                        p.b0_rng[1] > p.b0_rng[0]:
                    # bridge snapshot interleaved between chunk dispatches
                    sync.wait_ge(s_vd, 2 * RB)
                    sync.dma_start(
                        snapb[:],
                        et[RB % ED][:, p.b0_rng[0]:p.b0_rng[1]],
                    ).then_inc(s_o, 16)
            if p.a0_rng[1] > p.a0_rng[0]:
                sync.wait_ge(s_vd, 2 * RA)
                sync.dma_start(
                    snapa[:],
                    et[RA % ED][:, p.a0_rng[0]:p.a0_rng[1]],
                ).then_inc(s_o, 16)
            if p.fin_split:
                sync.wait_ge(s_f, p.fin_ops_early)
                sync.dma_start(fin[:, 0:p.fin_split],
                               fin_t[:, 0:p.fin_split]).then_inc(s_o, 16)
                sync.wait_ge(s_f, len(p.copies))
                sync.dma_start(fin[:, p.fin_split:],
                               fin_t[:, p.fin_split:]).then_inc(s_o, 16)
            else:
                sync.wait_ge(s_f, len(p.copies))
                sync.dma_start(fin[:, :], fin_t[:]).then_inc(s_o, 16)
            # d-segment end snapshot right after the last d-muls, then the
            # hop-A part once pool finishes round 16 (pipelines the DMA
            # fixed costs with the last pool mul)
            sync.wait_ge(s_vp, 2 * (HS - 1))
            sync.dma_start(
                snapf[:, zs - p.dm0:],
                et[(2 * (HS - 1)) % ED][:, zs:zs + YP]).then_inc(s_o, 16)

        @block.scalar
        def _(scalar):
            copies_by_round = {}
            for (r, e0, n, c0, fa) in p.copies:
                copies_by_round.setdefault(r, []).append((e0, n, c0, fa))
            if 0 in copies_by_round:
                scalar.wait_ge(s_xa, 16)
                for (e0, n, c0, fa) in copies_by_round[0]:
                    nc.scalar.copy(fin_t[:, fa:fa + n],
                                   rawa[0:64, c0:c0 + n]).then_inc(s_f, 1)
            for r in range(1, LB):
                hc = p.hop_chain(r)
                if hc is not None:
                    scalar.wait_ge(s_pp, r)
                    nc.scalar.copy(hbp[hc][:],
                                   psp[hc][:, :YP]).then_inc(s_hp, 1)
                if r in copies_by_round:
                    scalar.wait_ge(s_vd, 2 * (r - 1) + 1)
                    for (e0, n, c0, fa) in copies_by_round[r]:
                        nc.scalar.copy(
                            fin_t[:, fa:fa + n],
                            et[r % ED][0:64, c0:c0 + n]).then_inc(s_f, 1)
            # hop-B end snapshot (step 8 at round 15) from the idle ACT
            # queue so its HWDGE slot doesn't delay the final snapshots
            scalar.wait_ge(s_vp, 2 * (HS - 1) - 1)
            nc.scalar.dma_start(
                snapfb[:],
                et[(2 * (HS - 1) - 1) % ED][:, zs + YP:ncol]).then_inc(s_o, 16)
            # d-segment end snapshot from the ACT queue, split at the
            # chain boundary: part 1 ships while chain 1's last mul runs
            m15 = int(p.mids[1])
            scalar.wait_ge(s_vd, 2 * (SL - 1) - 1)
            nc.scalar.dma_start(
                snapf[:, 0:m15 - p.dm0],
                et[(SL - 1) % ED][:, p.dm0:m15]).then_inc(s_o, 16)
            scalar.wait_ge(s_vd, 2 * (SL - 1))
            nc.scalar.dma_start(
                snapf[:, m15 - p.dm0:zs - p.dm0],
                et[(SL - 1) % ED][:, m15:zs]).then_inc(s_o, 16)

        @block.tensor
        def _(tensor):
            def filler(n=1):
                for _ in range(n):
                    nc.tensor.matmul(psf[:], w[:], w[:, 0:128],
                                     start=True, stop=True)

            tensor.wait_ge(s_w, 16)
            filler(PRE_FILL)
            for r in range(1, LB):
                for cidx, (c0, n) in enumerate(drng(r)):
                    if r == 1:
                        if cidx == 0:
                            tensor.wait_ge(s_xa, 16)
                        mov = rawa[:, c0:c0 + n]
                    else:
                        tensor.wait_ge(s_vd, 2 * (r - 1) + cidx - 1)
                        mov = et[(r - 1) % ED][:, c0:c0 + n]
                    nc.tensor.matmul(
                        psd[cidx][:, :n], w[:], mov,
                        start=True, stop=True).then_inc(s_pd, 1)
                hc = p.hop_chain(r)
                if hc is not None:
                    cb = zs + hc * YP
                    if r <= 2:
                        if r == 1:
                            tensor.wait_ge(s_xa, 16)
                        mov = rawa[:, cb:cb + YP]
                    else:
                        tensor.wait_ge(s_vp, r - 2)
                        mov = et[(r - 2) % ED][:, cb:cb + YP]
                    nc.tensor.matmul(psp[hc][:, :YP], w[:], mov,
                                     start=True, stop=True).then_inc(s_pp, 1)
                filler(FILLERS)

        @block.vector
        def _(vector):
            for r in range(1, L):
                if r == CB[p.chunk_of[r]]:
                    chunk_arrived(vector, r)
                for cidx, (c0, n) in enumerate(drng(r)):
                    if cidx == 0:
                        if r >= ED and p.cum_copies[r - ED + 1] > \
                                p.cum_copies[r - ED]:
                            vector.wait_ge(s_f, int(p.cum_copies[r - ED + 1]))
                        if r - ED == RB and p.b0_rng[1] > p.b0_rng[0]:
                            vector.wait_ge(s_o, 16)
                    vector.wait_ge(s_pd, 2 * (r - 1) + cidx + 1)
                    nc.vector.tensor_mul(
                        et[r % ED][:, c0:c0 + n],
                        psd[cidx][:, :n],
                        xsl_d(r, c0, n)).then_inc(s_vd, 1)

        @block.gpsimd
        def _(gpsimd):
            for r in range(1, LB):
                hc = p.hop_chain(r)
                if hc is None:
                    continue
                if r == 1:
                    gpsimd.wait_ge(s_xh, 16)
                elif p.chunk_of[r] != p.chunk_of[r - 1]:
                    chunk_arrived(gpsimd, r)
                cb = zs + hc * YP
                gpsimd.wait_ge(s_hp, r)
                nc.gpsimd.tensor_mul(
                    et[r % ED][:, cb:cb + YP],
                    hbp[hc][:],
                    xsl_h(r, hc)).then_inc(s_vp, 1)

    return nc


def _postprocess(p, k, outs, x0_k, c):
    """Host math for core k: stitch offsets, read finals (float64)."""
    lx0 = np.log(np.maximum(np.asarray(x0_k, np.float64), 1e-300))
    lsb = np.log(np.maximum(np.asarray(outs["snapb"], np.float64), 1e-300))
    lsa = np.log(np.maximum(np.asarray(outs["snapa"], np.float64), 1e-300))
    lsf = np.log(np.maximum(np.asarray(outs["snapf"], np.float64), 1e-300))
    lsfb = np.log(np.maximum(np.asarray(outs["snapfb"], np.float64), 1e-300))
    lf = np.log(np.maximum(np.asarray(outs["fin"], np.float64), 1e-300))

    def vec(arr, rng0, col, half):
        return arr[64 * half:64 * half + 64, col - rng0]

    res = np.empty(BPC)
    for e in range(BPC):
        if int(p.tstar[k, e]) == 0:
            # alpha_0 = x_0 + origination, host-known exactly
            res[e] = vec(lx0, 0, int(p.ext_col[e]), 0).sum()
            continue
        segs = p.segs[k][e]
        r_e = int(p.re[k, e])
        A = 0.0
        for j in range(len(segs)):
            t0, steps, kind, col, half = segs[j]
            if kind == "B0":
                prev = vec(lsb, p.b0_rng[0], col, half) + SNAPB * c
            elif kind == "A0":
                prev = vec(lsa, p.a0_rng[0], col, half) + SNAPA * c
            elif kind == "D":
                prev = vec(lsf, p.dm0, col, half) + SL * c
            elif col < p.zs + YP:
                prev = vec(lsf, p.dm0, col, half) + HS * c
            else:
                prev = vec(lsfb, p.zs + YP, col, half) + HS * c
            # cur = raw init of the NEXT segment (or the extraction segment)
            if j + 1 < len(segs):
                ncol_, nhalf = segs[j + 1][3], segs[j + 1][4]
            else:
                ncol_, nhalf = int(p.ext_col[e]), 0
            cur = vec(lx0, 0, ncol_, nhalf)
            A += (prev - cur).mean()
        res[e] = lf[:, int(p.fincol[k, e])].sum() + 64.0 * (r_e * c + A)
    return res


def kernel(pad_x, transitions, origination, batch_sizes):
    from concourse.bass_utils import run_bass_kernel_spmd

    pad_x = np.asarray(pad_x)
    transitions = np.asarray(transitions)
    origination = np.asarray(origination)
    batch_sizes = np.asarray(batch_sizes)

    c = _c_step(transitions, pad_x)
    p = _plan(batch_sizes)
    xraw, wmat, x0 = _build_host_inputs(p, pad_x, transitions,
                                        origination, c)

    key = batch_sizes.tobytes()
    if key not in _CACHE:
        _CACHE[key] = _build_program(p)
    nc = _CACHE[key]

    in_maps = [{"xp": xraw[i], "wm": wmat} for i in range(NCORES)]
    out = run_bass_kernel_spmd(nc, in_maps, list(range(NCORES)))

    res = np.empty(B, np.float32)
    for k in range(NCORES):
        r = _postprocess(p, k, out.results[k], x0[k], c)
        for e in range(BPC):
            res[int(p.gidx[k, e])] = np.float32(r[e])
    return res


# revision 72
# speedup vs baseline: 1.0298x; 1.0298x over previous
"""Linear-chain CRF forward pass on 8 Trainium2 NeuronCores.

Reference recurrence (per batch element b):
    alpha_t[j] = x_t[j] + logsumexp_k(alpha_{t-1}[k] + trans[j,k])
    out[b] = sum_j alpha_{L_b - 1}[j]

Exp-space device formulation with a constant per-step log shift c folded
into the transition matrix:
    E_t = (Mc @ E_{t-1}) * X_t,  Mc[j,k] = exp(trans[j,k] - c),  X_t = exp(x_t)

The T=2048-step serial chain is cut per batch element into a chain of
segments with boundaries on multiples of 8; each segment evolves
independently from a raw X init (warmup W=2 inside the previous segment's
coverage; Birkhoff contraction converges the direction) and the per-segment
log offsets are recovered on the host by telescoping class-mean log-ratios
at the boundaries (each segment's end-state snapshot vs the host-known raw
init of the next segment).

Step 1 is computed on the host in fp32 (rawa ships E_1 = (Mc@X_0)*X_1),
so device round r advances step r+1 and only 16 rounds (0..15) run on
device.  Two segment populations share the schedule:
  - d-segments (16 steps, 1 step/round) live in the death-sorted front
    region: PE matmul -> PSUM fp32, DVE multiplies by X straight out of
    PSUM (2 chains).  Extraction segments and seg0 are always d-type.
  - hop-segments (8 steps, 1 step per TWO rounds) live in two YP-col
    suffix regions: PE matmul -> PSUM, ACT copies PSUM -> SBUF, GPSIMD
    (Pool) tensor_mul by X (neuronxcc rejects TensorScalarPtr on Pool).
    Chain A steps on even rounds, chain B on odd rounds; the 2-round
    cadence gives the PE->ACT->Pool serial path two full rounds, so it
    never stalls the d-chains.
  - Only live columns are shipped / computed: columns are death-sorted so
    the alive set each round is a suffix; extraction columns die at their
    extraction round (max 9), seg0 at its boundary snapshot (6 or 14).
  - Batch elements are grouped by extraction round and dealt round-robin
    to cores, so all cores share one extraction schedule and workload.
  - X = exp(x) is precomputed on the host, shipped bf16 in per-round
    variable-width slabs chunked through a 4-deep SBUF ring.
"""

from contextlib import ExitStack

import numpy as np

B, T, C = 256, 2048, 64
NCORES = 8
BPC = B // NCORES          # 32
SL = 16                    # d-segment steps
HS = 8                     # hop-segment steps (one step per 2 rounds)
W = 2                      # warmup rounds
L = SL                     # rounds 0..15; device round r computes step r+1
LB = SL                    # (step 1 = E1 is computed on the host)
RB = HS - W - 1            # seg0 bridge snapshot ROUND (step 6 at round 5)
RA = SL - W - 1            # seg0 snapshot ROUND (step 14 at round 13)
SNAPB = HS - W             # seg0 snapshot when first gap is 8
SNAPA = SL - W             # seg0 snapshot when first gap is 16
ED = 7                     # et ring depth
DCH = 2                    # rounds per X DMA chunk
RING = 4
NSX = 3
YP = 370                   # cols per hop chain (2 chains)
PRE_FILL = 5
FILLERS = 2

_CACHE = {}


def _c_step(transitions, pad_x):
    """Mean per-step growth of max_j alpha, from a short host simulation."""
    x = np.asarray(pad_x[:4], np.float64)
    tr = np.asarray(transitions, np.float64)
    a = x[:, 0, :]
    tot, n = 0.0, 0
    for t in range(1, 257):
        s = a[:, None, :] + tr[None, :, :]
        m = s.max(axis=2, keepdims=True)
        a_new = x[:, t, :] + np.log(np.exp(s - m).sum(axis=2)) + m[:, :, 0]
        tot += float((a_new.max(axis=1) - a.max(axis=1)).mean())
        n += 1
        a = a_new
    return tot / n


class _Plan:
    pass


def _plan(batch_sizes):
    bs = np.asarray(batch_sizes).astype(np.int64)
    p = _Plan()

    # --- assignment: group by r_e, round-robin to cores -------------------
    info = []
    for b in range(B):
        ts = int(bs[b]) - 1
        if ts < HS:
            r_e, bm = ts, 0
        else:
            bm = (ts // HS) * HS
            r_e = ts - bm + W
        info.append((r_e, ts, b, bm))
    info.sort()
    p.gidx = np.zeros((NCORES, BPC), np.int64)
    p.re = np.zeros((NCORES, BPC), np.int64)
    p.tstar = np.zeros((NCORES, BPC), np.int64)
    p.bm = np.zeros((NCORES, BPC), np.int64)
    for rank, (r_e, ts, b, bm) in enumerate(info):
        k, e = rank % NCORES, rank // NCORES
        p.gidx[k, e] = b
        p.re[k, e] = r_e
        p.tstar[k, e] = ts
        p.bm[k, e] = bm
    p.cre = np.maximum(p.re - 1, 0)      # device copy round per event
    ext_death = p.cre.max(axis=0)
    assert int(ext_death.max()) <= HS + W - 1

    # --- per-element segment gap lists (8s and 16s), hop quota ------------
    QHOP = 4 * YP              # hop half-slots per core (2 chains x 2 halves)
    p.gaps = [[None] * BPC for _ in range(NCORES)]
    counts = {"B0": np.zeros(NCORES, np.int64),
              "A0": np.zeros(NCORES, np.int64),
              "D": np.zeros(NCORES, np.int64),
              "H": np.zeros(NCORES, np.int64)}
    for k in range(NCORES):
        q = QHOP
        # big elements first so quota parity always resolves
        order = sorted(range(BPC), key=lambda e: -int(p.bm[k, e]))
        for e in order:
            G = int(p.bm[k, e]) // HS
            # seg0's gap is free (B0/A0 column); only mid 8-gaps use hop
            # slots, so up to q+1 eights fit.  Parity: n8 must match G.
            n8 = min(G, q + 1)
            if (n8 - G) % 2:
                n8 -= 1
            if n8 < G % 2:
                n8 = G % 2
            q -= max(n8 - 1, 0)
            n16 = (G - n8) // 2
            # gap list in chain order: one 8 first if any (seg0 -> B0)
            if n8 >= 1:
                gaps = [8] * n8 + [16] * n16
            else:
                gaps = [16] * n16
            assert sum(gaps) == int(p.bm[k, e])
            p.gaps[k][e] = gaps
            if gaps:
                counts["B0" if gaps[0] == 8 else "A0"][k] += 1
                counts["H"][k] += (n8 - 1) if n8 >= 1 else 0
                counts["D"][k] += n16 if n8 >= 1 else n16 - 1
        assert q >= -1
    NB0 = int(max((int(n) + 1) // 2 for n in counts["B0"]))
    NA0 = int(max((int(n) + 1) // 2 for n in counts["A0"]))
    NDM = int(max((int(n) + 1) // 2 for n in counts["D"])) + 1

    # --- global death-sorted d-region columns -----------------------------
    cols = [(int(ext_death[e]), 0, e) for e in range(BPC)]
    cols += [(RB, 1, i) for i in range(NB0)]
    cols.sort()
    cols += [(RA, 2, i) for i in range(NA0)]
    cols += [(SL - 1, 3, i) for i in range(NDM)]
    p.zs = len(cols)
    p.ncol = p.zs + 2 * YP
    p.col_death = np.array([cc[0] for cc in cols], np.int64)
    assert np.all(np.diff(p.col_death) >= 0)
    p.ext_col = np.zeros(BPC, np.int64)
    b0_cols, a0_cols = [], []
    dm0 = None
    for ci, (_, cls, ident) in enumerate(cols):
        if cls == 0:
            p.ext_col[ident] = ci
        elif cls == 1:
            b0_cols.append(ci)
        elif cls == 2:
            a0_cols.append(ci)
        elif dm0 is None:
            dm0 = ci
    p.b0_rng = (b0_cols[0], b0_cols[-1] + 1) if b0_cols else (0, 0)
    p.a0_rng = (a0_cols[0], a0_cols[-1] + 1) if a0_cols else (0, 0)
    p.dm0 = dm0 if dm0 is not None else p.zs

    p.d = np.array([int(np.searchsorted(p.col_death, r))
                    for r in range(L + 1)], np.int64)

    # --- per-core slot assignment ----------------------------------------
    # segs[k][e] = list of (t0, steps, kind, col, half); ext is separate
    p.segs = [[None] * BPC for _ in range(NCORES)]
    for k in range(NCORES):
        it_b0 = iter([(c, h) for c in b0_cols for h in (0, 1)])
        it_a0 = iter([(c, h) for c in a0_cols for h in (0, 1)])
        it_d = iter([(c, h) for c in range(p.dm0, p.zs) for h in (0, 1)])
        it_h = iter([(c, h) for c in range(p.zs, p.ncol) for h in (0, 1)])
        for e in range(BPC):
            gaps = p.gaps[k][e]
            segs = []
            b_cum = 0
            for j, g in enumerate(gaps):
                t0 = 0 if j == 0 else b_cum - W
                if j == 0:
                    kind = "B0" if g == 8 else "A0"
                    col, half = next(it_b0 if g == 8 else it_a0)
                    steps = SNAPB if g == 8 else SNAPA
                elif g == 16:
                    kind, (col, half), steps = "D", next(it_d), SL
                else:
                    kind, (col, half), steps = "H", next(it_h), HS
                segs.append((t0, steps, kind, col, half))
                b_cum += g
            p.segs[k][e] = segs

    # --- extraction copy ops ---------------------------------------------
    re_min = p.cre.min(axis=0)
    re_max = p.cre.max(axis=0)
    p.copies = []
    fin_off = 0
    for r in range(L):
        es = [e for e in range(BPC) if re_min[e] <= r <= re_max[e]]
        if not es:
            continue
        runs = []
        for e in es:
            cc = int(p.ext_col[e])
            if runs and e == runs[-1][1] + 1 and cc == runs[-1][3] + 1:
                runs[-1][1] = e
                runs[-1][3] = cc
            else:
                runs.append([e, e, cc, cc])
        for (e0, e1, c0, _c1) in runs:
            n = e1 - e0 + 1
            p.copies.append((r, e0, n, c0, fin_off))
            fin_off += n
    p.nfin = fin_off
    p.fincol = np.zeros((NCORES, BPC), np.int64)
    for k in range(NCORES):
        for e in range(BPC):
            r = int(p.cre[k, e])
            for (rr, e0, n, c0, fa) in p.copies:
                if rr == r and e0 <= e < e0 + n:
                    p.fincol[k, e] = fa + (e - e0)
                    break
            else:
                raise AssertionError("no copy op for event")
    p.cum_copies = np.zeros(L + 1, np.int64)
    for r in range(L):
        p.cum_copies[r + 1] = p.cum_copies[r] + sum(
            1 for (rr, *_x) in p.copies if rr == r)

    # --- chain geometry ---------------------------------------------------
    # fixed split point: chain 1 = [MID, zs) constant; chain 0 absorbs the
    # shrinking alive front [d(r), MID) (ranges must nest across rounds)
    MID = (int(p.d[L - 1]) + p.zs) // 2
    p.mids = np.zeros(L, np.int64)
    for r in range(1, L):
        lo = int(p.d[r])
        assert MID - lo <= 512 and p.zs - MID <= 512
        p.mids[r] = MID
    for (r, e0, n, c0, fa) in p.copies:
        if r >= 1:
            assert c0 + n <= int(p.mids[r])
    assert p.b0_rng[1] <= int(p.mids[RB]) or p.b0_rng[1] == 0

    # --- X slab layout ----------------------------------------------------
    # round r slab: d-part [d(r), zs) + hop part (512 cols) for the active
    # chain: A steps on even rounds (2..16, step r/2), B on odd (1..15,
    # step (r+1)/2).  B's step 8 lands at round 15 so its end snapshot
    # ships during round 16.
    def hop_chain(r):
        if 2 <= r <= 2 * (HS - 1) and r % 2 == 0:
            return 0
        if 1 <= r <= 2 * (HS - 1) - 1 and r % 2 == 1:
            return 1
        return None
    p.hop_chain = hop_chain
    p.O = np.zeros(LB + 1, np.int64)
    p.O[1] = p.ncol                      # rawa
    for r in range(1, LB):
        wdt = p.zs - int(p.d[r])
        if hop_chain(r) is not None:
            wdt += YP
        p.O[r + 1] = p.O[r] + wdt
    p.ntot = int(p.O[LB])
    CB = [1, 2, 3]
    while CB[-1] < LB:
        CB.append(min(CB[-1] + DCH, LB))
    p.CB = CB
    p.nchunk = len(CB) - 1
    p.chunk_of = [0] * LB
    for kk in range(p.nchunk):
        for r in range(CB[kk], CB[kk + 1]):
            p.chunk_of[r] = kk
    p.maxchunkw = max(int(p.O[CB[kk + 1]] - p.O[CB[kk]])
                      for kk in range(p.nchunk))

    # early/late fin split
    p.fin_split = 0
    p.fin_ops_early = 0
    for (r, e0, n, c0, fa) in p.copies:
        if r <= HS - 2:
            p.fin_ops_early += 1
            p.fin_split = max(p.fin_split, fa + n)
    if p.fin_split > p.nfin - 4:
        p.fin_split = 0
    return p


def _build_host_inputs(p, pad_x, transitions, origination, c):
    import ml_dtypes
    mc = np.exp(np.asarray(transitions, np.float64) - c).astype(np.float32)
    wmat = np.zeros((128, 128), ml_dtypes.bfloat16)
    wmat[:64, :64] = mc.T.astype(ml_dtypes.bfloat16)
    wmat[64:, 64:] = mc.T.astype(ml_dtypes.bfloat16)

    x0 = np.asarray(pad_x, np.float32).copy()
    x0[:, 0, :] += np.asarray(origination, np.float32)[None, :]

    ncol, zs = p.ncol, p.zs
    xraw = np.empty((NCORES, 128, p.ntot), ml_dtypes.bfloat16)
    x0s = np.empty((NCORES, 128, ncol), np.float32)
    for k in range(NCORES):
        t0s = np.full((2, ncol), -10 ** 9, np.int64)
        bofs = np.zeros((2, ncol), np.int64)
        for e in range(BPC):
            gb = int(p.gidx[k, e])
            ts = int(p.tstar[k, e])
            t0s[0, int(p.ext_col[e])] = 0 if ts < HS else int(p.bm[k, e]) - W
            bofs[0, int(p.ext_col[e])] = gb
            for (t0, steps, kind, col, half) in p.segs[k][e]:
                t0s[half, col] = t0
                bofs[half, col] = gb
        # device steps per column: d-region cols see step r at round r;
        # hop cols see step s at round 2s (A) / 2s+1 (B).
        xfull = np.empty((128, SL + 1, ncol), np.float32)
        for hh in range(2):
            t_idx = t0s[hh][:, None] + np.arange(SL + 1)[None, :]
            valid = (t_idx >= 0) & (t_idx < T) & (t0s[hh][:, None] > -10**8)
            t_clip = np.clip(t_idx, 0, T - 1)
            blk = x0[bofs[hh][:, None], t_clip, :]
            blk = np.where(valid[:, :, None], blk, 0.0)
            xfull[64 * hh:64 * hh + 64] = np.exp(blk).transpose(2, 1, 0)
        # host-computed step 1: E1 = (Mc @ X0) * X1 per 64-class half
        e1 = np.empty((128, ncol), np.float32)
        for hh in range(2):
            sl_ = slice(64 * hh, 64 * hh + 64)
            e1[sl_] = (mc @ xfull[sl_, 0, :]) * xfull[sl_, 1, :]
        out = xraw[k]
        out[:, 0:ncol] = e1.astype(ml_dtypes.bfloat16)
        x0s[k] = xfull[:, 0, :]
        for r in range(1, LB):
            o = int(p.O[r])
            d = int(p.d[r])
            out[:, o:o + zs - d] = \
                xfull[:, r + 1, d:zs].astype(ml_dtypes.bfloat16)
            o += zs - d
            hc = p.hop_chain(r)
            if hc is not None:
                s = (r // 2 if hc == 0 else (r + 1) // 2) + 1
                cb = zs + hc * YP
                out[:, o:o + YP] = \
                    xfull[:, s, cb:cb + YP].astype(ml_dtypes.bfloat16)
    return xraw, wmat, x0s


def _build_program(p):
    import concourse.bass as bass
    from concourse import mybir

    dt = mybir.dt
    ncol, zs = p.ncol, p.zs
    CB = p.CB

    nc = bass.Bass()
    xp = nc.declare_dram_parameter("xp", [128, p.ntot], dt.bfloat16, False)
    wm = nc.declare_dram_parameter("wm", [128, 128], dt.bfloat16, False)
    snapb = nc.declare_dram_parameter(
        "snapb", [128, max(p.b0_rng[1] - p.b0_rng[0], 1)], dt.bfloat16, True)
    snapa = nc.declare_dram_parameter(
        "snapa", [128, max(p.a0_rng[1] - p.a0_rng[0], 1)], dt.bfloat16, True)
    snapf = nc.declare_dram_parameter(
        "snapf", [128, zs + YP - p.dm0], dt.bfloat16, True)
    snapfb = nc.declare_dram_parameter(
        "snapfb", [128, YP], dt.bfloat16, True)
    fin = nc.declare_dram_parameter("fin", [64, p.nfin], dt.bfloat16, True)

    with ExitStack() as ctx:
        def sb(name, shape, d):
            return ctx.enter_context(nc.sbuf_tensor(name, shape, d))
        w = sb("w", [128, 128], dt.bfloat16)
        rawa = sb("rawa", [128, ncol], dt.bfloat16)
        raw = [sb(f"raw{i}", [128, p.maxchunkw], dt.bfloat16)
               for i in range(RING)]
        et = [sb(f"et{i}", [128, ncol], dt.bfloat16) for i in range(ED)]
        hbp = [sb(f"hbp{i}", [128, YP], dt.bfloat16) for i in range(2)]
        fin_t = sb("fin_t", [64, p.nfin], dt.bfloat16)
        psd = [ctx.enter_context(
            nc.psum_tensor(f"psd{cidx}", [128, 512], dt.float32))
            for cidx in range(2)]
        psp = [ctx.enter_context(
            nc.psum_tensor(f"psp{i}", [128, 512], dt.float32))
            for i in range(2)]
        psf = ctx.enter_context(nc.psum_tensor("psf", [128, 128], dt.float32))
        s_w = ctx.enter_context(nc.semaphore("s_w"))
        s_xa = ctx.enter_context(nc.semaphore("s_xa"))
        s_xh = ctx.enter_context(nc.semaphore("s_xh"))
        s_x = tuple(ctx.enter_context(nc.semaphore(f"s_x{i}"))
                    for i in range(NSX))
        s_pd = ctx.enter_context(nc.semaphore("s_pd"))
        s_pp = ctx.enter_context(nc.semaphore("s_pp"))
        s_hp = ctx.enter_context(nc.semaphore("s_hp"))
        s_vd = ctx.enter_context(nc.semaphore("s_vd"))
        s_vp = ctx.enter_context(nc.semaphore("s_vp"))
        s_f = ctx.enter_context(nc.semaphore("s_f"))
        s_o = ctx.enter_context(nc.semaphore("s_o"))
        block = ctx.enter_context(nc.Block())

        def drng(r):
            lo, m = int(p.d[r]), int(p.mids[r])
            return ((lo, m - lo), (m, zs - m))

        def xsl_d(r, c0, n):
            kk = p.chunk_of[r]
            off = int(p.O[r] - p.O[CB[kk]]) + (c0 - int(p.d[r]))
            return raw[kk % RING][:, off:off + n]

        def xsl_h(r, hc):
            kk = p.chunk_of[r]
            off = int(p.O[r] - p.O[CB[kk]]) + \
                ((zs - int(p.d[r])) if r <= SL else 0)
            return raw[kk % RING][:, off:off + YP]

        def chunk_arrived(eng, r):
            kk = p.chunk_of[r]
            eng.wait_ge(s_x[kk % NSX], 16 * (kk // NSX + 1))

        @block.sync
        def _(sync):
            sync.dma_start(w[:], wm[:, :]).then_inc(s_w, 16)
            sync.dma_start(rawa[:], xp[:, 0:ncol]).then_inc(s_xa, 16)
            # chunk 0 d-part first; its hop part is deferred past chunk 1
            # (round-1 pool work has 2 rounds of slack, round-2 d does not)
            c0a, c0b = int(p.O[1]), int(p.O[2])
            wd1 = zs - int(p.d[1])
            sync.dma_start(raw[0][:, :wd1],
                           xp[:, c0a:c0a + wd1]).then_inc(s_x[0], 16)
            o12, o13 = int(p.O[2]), int(p.O[3])
            sync.dma_start(raw[1][:, :o13 - o12],
                           xp[:, o12:o13]).then_inc(s_x[1], 16)
            sync.dma_start(raw[0][:, wd1:c0b - c0a],
                           xp[:, c0a + wd1:c0b]).then_inc(s_xh, 16)
            for kk in range(2, p.nchunk):
                if kk >= RING:
                    r_last = CB[kk - RING + 1] - 1
                    sync.wait_ge(s_vd, 2 * min(r_last, SL - 1))
                    sync.wait_ge(s_vp, min(r_last, 2 * (HS - 1)))
                if kk >= NSX:
                    sync.wait_ge(s_x[kk % NSX], 16 * (kk // NSX))
                o0, o1 = int(p.O[CB[kk]]), int(p.O[CB[kk + 1]])
                sync.dma_start(
                    raw[kk % RING][:, :o1 - o0], xp[:, o0:o1],
                ).then_inc(s_x[kk % NSX], 16)
                if CB[kk] <= RB + 6 < CB[kk + 1] and \
                        p.b0_rng[1] > p.b0_rng[0]:
                    # bridge snapshot interleaved between chunk dispatches
                    sync.wait_ge(s_vd, 2 * RB)
                    sync.dma_start(
                        snapb[:],
                        et[RB % ED][:, p.b0_rng[0]:p.b0_rng[1]],
                    ).then_inc(s_o, 16)
            if p.a0_rng[1] > p.a0_rng[0]:
                sync.wait_ge(s_vd, 2 * RA)
                sync.dma_start(
                    snapa[:],
                    et[RA % ED][:, p.a0_rng[0]:p.a0_rng[1]],
                ).then_inc(s_o, 16)
            if p.fin_split:
                sync.wait_ge(s_f, p.fin_ops_early)
                sync.dma_start(fin[:, 0:p.fin_split],
                               fin_t[:, 0:p.fin_split]).then_inc(s_o, 16)
                sync.wait_ge(s_f, len(p.copies))
                sync.dma_start(fin[:, p.fin_split:],
                               fin_t[:, p.fin_split:]).then_inc(s_o, 16)
            else:
                sync.wait_ge(s_f, len(p.copies))
                sync.dma_start(fin[:, :], fin_t[:]).then_inc(s_o, 16)
            # d-segment end snapshot right after the last d-muls, then the
            # hop-A part once pool finishes round 16 (pipelines the DMA
            # fixed costs with the last pool mul)
            sync.wait_ge(s_vp, 2 * (HS - 1))
            sync.dma_start(
                snapf[:, zs - p.dm0:],
                et[(2 * (HS - 1)) % ED][:, zs:zs + YP]).then_inc(s_o, 16)

        @block.scalar
        def _(scalar):
            copies_by_round = {}
            for (r, e0, n, c0, fa) in p.copies:
                copies_by_round.setdefault(r, []).append((e0, n, c0, fa))
            if 0 in copies_by_round:
                scalar.wait_ge(s_xa, 16)
                for (e0, n, c0, fa) in copies_by_round[0]:
                    nc.scalar.copy(fin_t[:, fa:fa + n],
                                   rawa[0:64, c0:c0 + n]).then_inc(s_f, 1)
            for r in range(1, LB):
                hc = p.hop_chain(r)
                if hc is not None:
                    scalar.wait_ge(s_pp, r)
                    nc.scalar.copy(hbp[hc][:],
                                   psp[hc][:, :YP]).then_inc(s_hp, 1)
                if r in copies_by_round:
                    scalar.wait_ge(s_vd, 2 * (r - 1) + 1)
                    for (e0, n, c0, fa) in copies_by_round[r]:
                        nc.scalar.copy(
                            fin_t[:, fa:fa + n],
                            et[r % ED][0:64, c0:c0 + n]).then_inc(s_f, 1)
            # hop-B end snapshot (step 8 at round 15) from the idle ACT
            # queue so its HWDGE slot doesn't delay the final snapshots
            scalar.wait_ge(s_vp, 2 * (HS - 1) - 1)
            nc.scalar.dma_start(
                snapfb[:],
                et[(2 * (HS - 1) - 1) % ED][:, zs + YP:ncol]).then_inc(s_o, 16)
            # d-segment end snapshot from the ACT queue: its HWDGE slot
            # overlaps the SP queue's fin/hop-A generations
            scalar.wait_ge(s_vd, 2 * (SL - 1))
            nc.scalar.dma_start(
                snapf[:, 0:zs - p.dm0],
                et[(SL - 1) % ED][:, p.dm0:zs]).then_inc(s_o, 16)

        @block.tensor
        def _(tensor):
            def filler(n=1):
                for _ in range(n):
                    nc.tensor.matmul(psf[:], w[:], w[:, 0:128],
                                     start=True, stop=True)

            tensor.wait_ge(s_w, 16)
            filler(PRE_FILL)
            for r in range(1, LB):
                for cidx, (c0, n) in enumerate(drng(r)):
                    if r == 1:
                        if cidx == 0:
                            tensor.wait_ge(s_xa, 16)
                        mov = rawa[:, c0:c0 + n]
                    else:
                        tensor.wait_ge(s_vd, 2 * (r - 1) + cidx - 1)
                        mov = et[(r - 1) % ED][:, c0:c0 + n]
                    nc.tensor.matmul(
                        psd[cidx][:, :n], w[:], mov,
                        start=True, stop=True).then_inc(s_pd, 1)
                hc = p.hop_chain(r)
                if hc is not None:
                    cb = zs + hc * YP
                    if r <= 2:
                        if r == 1:
                            tensor.wait_ge(s_xa, 16)
                        mov = rawa[:, cb:cb + YP]
                    else:
                        tensor.wait_ge(s_vp, r - 2)
                        mov = et[(r - 2) % ED][:, cb:cb + YP]
                    nc.tensor.matmul(psp[hc][:, :YP], w[:], mov,
                                     start=True, stop=True).then_inc(s_pp, 1)
                filler(FILLERS)

        @block.vector
        def _(vector):
            for r in range(1, L):
                if r == CB[p.chunk_of[r]]:
                    chunk_arrived(vector, r)
                for cidx, (c0, n) in enumerate(drng(r)):
                    if cidx == 0:
                        if r >= ED and p.cum_copies[r - ED + 1] > \
                                p.cum_copies[r - ED]:
                            vector.wait_ge(s_f, int(p.cum_copies[r - ED + 1]))
                        if r - ED == RB and p.b0_rng[1] > p.b0_rng[0]:
                            vector.wait_ge(s_o, 16)
                    vector.wait_ge(s_pd, 2 * (r - 1) + cidx + 1)
                    nc.vector.tensor_mul(
                        et[r % ED][:, c0:c0 + n],
                        psd[cidx][:, :n],
                        xsl_d(r, c0, n)).then_inc(s_vd, 1)

        @block.gpsimd
        def _(gpsimd):
            for r in range(1, LB):
                hc = p.hop_chain(r)
                if hc is None:
                    continue
                if r == 1:
                    gpsimd.wait_ge(s_xh, 16)
                elif p.chunk_of[r] != p.chunk_of[r - 1]:
                    chunk_arrived(gpsimd, r)
                cb = zs + hc * YP
                gpsimd.wait_ge(s_hp, r)
                nc.gpsimd.tensor_mul(
                    et[r % ED][:, cb:cb + YP],
                    hbp[hc][:],
                    xsl_h(r, hc)).then_inc(s_vp, 1)

    return nc


def _postprocess(p, k, outs, x0_k, c):
    """Host math for core k: stitch offsets, read finals (float64)."""
    lx0 = np.log(np.maximum(np.asarray(x0_k, np.float64), 1e-300))
    lsb = np.log(np.maximum(np.asarray(outs["snapb"], np.float64), 1e-300))
    lsa = np.log(np.maximum(np.asarray(outs["snapa"], np.float64), 1e-300))
    lsf = np.log(np.maximum(np.asarray(outs["snapf"], np.float64), 1e-300))
    lsfb = np.log(np.maximum(np.asarray(outs["snapfb"], np.float64), 1e-300))
    lf = np.log(np.maximum(np.asarray(outs["fin"], np.float64), 1e-300))

    def vec(arr, rng0, col, half):
        return arr[64 * half:64 * half + 64, col - rng0]

    res = np.empty(BPC)
    for e in range(BPC):
        if int(p.tstar[k, e]) == 0:
            # alpha_0 = x_0 + origination, host-known exactly
            res[e] = vec(lx0, 0, int(p.ext_col[e]), 0).sum()
            continue
        segs = p.segs[k][e]
        r_e = int(p.re[k, e])
        A = 0.0
        for j in range(len(segs)):
            t0, steps, kind, col, half = segs[j]
            if kind == "B0":
                prev = vec(lsb, p.b0_rng[0], col, half) + SNAPB * c
            elif kind == "A0":
                prev = vec(lsa, p.a0_rng[0], col, half) + SNAPA * c
            elif kind == "D":
                prev = vec(lsf, p.dm0, col, half) + SL * c
            elif col < p.zs + YP:
                prev = vec(lsf, p.dm0, col, half) + HS * c
            else:
                prev = vec(lsfb, p.zs + YP, col, half) + HS * c
            # cur = raw init of the NEXT segment (or the extraction segment)
            if j + 1 < len(segs):
                ncol_, nhalf = segs[j + 1][3], segs[j + 1][4]
            else:
                ncol_, nhalf = int(p.ext_col[e]), 0
            cur = vec(lx0, 0, ncol_, nhalf)
            A += (prev - cur).mean()
        res[e] = lf[:, int(p.fincol[k, e])].sum() + 64.0 * (r_e * c + A)
    return res


def kernel(pad_x, transitions, origination, batch_sizes):
    from concourse.bass_utils import run_bass_kernel_spmd

    pad_x = np.asarray(pad_x)
    transitions = np.asarray(transitions)
    origination = np.asarray(origination)
    batch_sizes = np.asarray(batch_sizes)

    c = _c_step(transitions, pad_x)
    p = _plan(batch_sizes)
    xraw, wmat, x0 = _build_host_inputs(p, pad_x, transitions,
                                        origination, c)

    key = batch_sizes.tobytes()
    if key not in _CACHE:
        _CACHE[key] = _build_program(p)
    nc = _CACHE[key]

    in_maps = [{"xp": xraw[i], "wm": wmat} for i in range(NCORES)]
    out = run_bass_kernel_spmd(nc, in_maps, list(range(NCORES)))

    res = np.empty(B, np.float32)
    for k in range(NCORES):
        r = _postprocess(p, k, out.results[k], x0[k], c)
        for e in range(BPC):
            res[int(p.gidx[k, e])] = np.float32(r[e])
    return res
